# revision 18
# baseline (speedup 1.0000x reference)
"""Bass/Trainium2 kernel for framed 2-layer BiLSTM (nn_BLSTM).

Data-parallel over the 80 framed sequences: 10 per core on 8 NeuronCores.

Restructured recurrence (vs the staged-projection baseline): the input
projection Wx*x_t, bias, and recurrent Wh*h_{t-1} all accumulate directly
into one [128 x 240] PSUM tile per direction per step (transposed layout:
partitions = gate channels mod 128, columns = m-tile x sequence). All
matmuls are fp8 DoubleRow (K=256 per instruction). Gate order is host-
reordered to [i, f, g, o] with the g block pre-scaled by 2 so ONE sigmoid
over [i,f,2g] yields sigma(i), sigma(f), and sigma(2g) = (tanh(g)+1)/2;
tanh(g) is then reconstructed on the DVE via c = sf*c + 2*(si*s2g) - si.
No xw staging in DRAM, no PE injects, no projection drains.

Layer-1 consumes layer-0's fp8 h buffer directly as its Wx input. The
final linear consumes a bf16 copy of h1 streamed through DRAM (SBUF
chunks -> DRAM -> row tiles), keeping fp8 out of the output path.
"""
import sys
import numpy as np

sys.path.insert(0, "/opt/trn_rl_repo")

import ml_dtypes  # noqa: E402
import concourse.bass as bass  # noqa: E402
import concourse.mybir as mybir  # noqa: E402
from concourse import bacc  # noqa: E402
from concourse.tile import TileContext  # noqa: E402
from concourse.bass_utils import run_bass_kernel_spmd  # noqa: E402

# ---- custom DVE ops: fused tanh(g) reconstruction and a 3-op tanh(c) ----
# tanh(x) ~ clamp(x*(x^4+105x^2+945) / (15*((x^2+14)^2-133))), Pade(5,4)
# with a bitwise-NOT reciprocal seed + one Newton step (max abs err ~3.4e-3).
import concourse.dve_ops as _dvo  # noqa: E402
from concourse.dve_spec import (  # noqa: E402
    Spec, Src0, Src1, C0, C1, C2, One, AluOp, Bin, sq, maxx, minn, lower,
    _has_src1)
from concourse.dve_uop import DveOpSpec  # noqa: E402


def _register_dve_op(name, body, reference):
    for op in _dvo.OPS:
        if op.name == name:
            return op
    spec = Spec(body=body, reference=reference)
    row = max(_dvo._SUB_OPCODE_FOR_NAME.values()) + 1
    assert row < 0x20
    _dvo._SUB_OPCODE_FOR_NAME[name] = row
    sha = DveOpSpec(name=name, opcode=row, uops=lower(spec, ver="v3"),
                    rd1_en=_has_src1(spec)).sha("v3")
    op = _dvo.DveOp(name, spec, subdim=False, uops_sha={"v3": sha})
    _dvo.OPS.append(op)
    _dvo.CUSTOM_DVE_SPECS[name] = spec
    return op


def _bnot(v):
    return (~v.view(np.int32)).view(np.float32)


_x2 = sq(Src0)
_dq = sq(_x2 + C0) - C1
OP_TANH_SEED = _register_dve_op(
    "TANH_SEED_ANT", Bin(AluOp.BITWISE_NOT, _dq, _dq) * C2,
    lambda in0, in1, c0, c1, c2:
    _bnot(((in0 * in0 + c0) ** 2 - c1).astype(np.float32)) * c2)
OP_TANH_NR = _register_dve_op(
    "TANH_NR_ANT", Src1 * (C2 - (sq(sq(Src0) + C0) - C1) * Src1),
    lambda in0, in1, c0, c1, c2:
    in1 * (c2 - ((in0 * in0 + c0) ** 2 - c1) * in1))
OP_TANH_NUM = _register_dve_op(
    "TANH_NUM_ANT", (((sq(Src0) + C0) * sq(Src0) + C1) * Src0) * Src1 * C2,
    lambda in0, in1, c0, c1, c2:
    ((in0 * in0 + c0) * (in0 * in0) + c1) * in0 * in1 * c2)
OP_HMUL_CLAMP = _register_dve_op(
    "HMUL_CLAMP_ANT", Src0 * minn(maxx(Src1, C0), One),
    lambda in0, in1, c0, c1, c2: in0 * np.minimum(np.maximum(in1, c0), 1.0))
OP_AB2_SUB = _register_dve_op(
    "AB2_SUB_ANT", ((Src0 + Src0) - One) * Src1,
    lambda in0, in1, c0, c1, c2: (2.0 * in0 - 1.0) * in1)

F32 = mybir.dt.float32
BF16 = mybir.dt.bfloat16
FP8 = mybir.dt.float8e4
BF16_NP = ml_dtypes.bfloat16
FP8_NP = ml_dtypes.float8_e4m3
DR = mybir.MatmulPerfMode.DoubleRow

DIM = 768
H = 768
G = 4 * H            # 3072, gate order reordered to [i, f, 2g, o]
B, T = 4, 2000
WIDTH, STRIDE = 200, 100
NFR = 20             # frames per batch element
NSEQ = B * NFR       # 80
NCORES = 8
SEQ_PC = NSEQ // NCORES   # 10
ROWS = SEQ_PC * WIDTH     # 2000 rows per core, row = t*10 + s (t-major)
MT = (ROWS + 127) // 128  # 16 row m-tiles for the final linear
NM = 24                   # gate m-tiles

SIG = mybir.ActivationFunctionType.Sigmoid
TANH = mybir.ActivationFunctionType.Tanh
MUL = mybir.AluOpType.mult
ADD = mybir.AluOpType.add
SUB = mybir.AluOpType.subtract

_CACHE = {}


def _mrows(m):
    return min(128, ROWS - m * 128)


def _build_program():
    nc = bacc.Bacc("TRN2", target_bir_lowering=False, debug=False,
                   num_devices=NCORES)

    xT_d = nc.declare_dram_parameter("xT", [DIM, ROWS], FP8, isOutput=False)
    wx0_d = nc.declare_dram_parameter("wx0", [2, DIM, G], FP8, isOutput=False)
    wh0_d = nc.declare_dram_parameter("wh0", [2, H, G], FP8, isOutput=False)
    wx1_d = nc.declare_dram_parameter("wx1", [2, 2 * H, G], FP8,
                                      isOutput=False)
    wh1_d = nc.declare_dram_parameter("wh1", [2, H, G], FP8, isOutput=False)
    bT_d = nc.declare_dram_parameter("bT", [2, 2, 12, 2, 128], FP8,
                                     isOutput=False)
    mask_d = nc.declare_dram_parameter("mask", [12, 2, NM * 10], FP8,
                                       isOutput=False)
    linw_d = nc.declare_dram_parameter("linw", [2 * H, DIM], BF16,
                                       isOutput=False)
    linb_d = nc.declare_dram_parameter("linb", [DIM], F32, isOutput=False)
    out_d = nc.declare_dram_parameter("out", [ROWS, DIM], F32, isOutput=True)

    # bf16 h1 staging for the final linear: [p, dir, k, row]
    h1_d = nc.dram_tensor("h1s", [128, 2, 6, ROWS], BF16)

    with TileContext(nc) as tc:
        with tc.tile_pool(name="const", bufs=1) as constp, \
             tc.tile_pool(name="hbp", bufs=1) as hbp, \
             tc.tile_pool(name="cstp", bufs=1) as cstp:
            ones = constp.tile([1, 128], F32)
            nc.vector.memset(ones[:], 1.0)
            bT_sb = [constp.tile([12, 2, 2, 128], FP8, name=f"bT{l}")
                     for l in range(2)]
            for l in range(2):
                for d in range(2):
                    nc.sync.dma_start(bT_sb[l][:, d], bT_d[l, d])
            mask_sb = constp.tile([12, 2, NM * 10], FP8)
            nc.sync.dma_start(mask_sb[:], mask_d[:])

            hT0 = hbp.tile([128, 12, ROWS], FP8, name="hT0")

            # ---- one recurrence layer, both dirs interleaved ----
            # xsrc(d, k2, tt) -> [128, 2, 10] moving AP for the input
            # projection; kx2 = number of fp8-DoubleRow K pairs of the input.
            def recur(l, wx_sb, wh_sb, bsb, xsrc, kx2, hT_out, stag=False,
                      post_iter=None):
                with tc.tile_pool(name=f"gp{l}", bufs=2, space="PSUM") as gpp,\
                     tc.tile_pool(name=f"sg{l}", bufs=2) as sgp, \
                     tc.tile_pool(name=f"sc{l}", bufs=2) as scp, \
                     tc.tile_pool(name=f"st{l}", bufs=2) as stp:
                    c = [cstp.tile([128, 60], F32, name=f"c{l}{d}")
                         for d in range(2)]
                    for d in range(2):
                        nc.vector.memset(c[d][:], 0.0)
                    ps_cur = [None, None]
                    ps_nxt = [None, None]
                    stg = [None, None]

                    def prefill(d, t):
                        tt = t if d == 0 else WIDTH - 1 - t
                        ps = gpp.tile([128, 240], F32, tag=f"g{d}",
                                      name=f"g{l}{d}")
                        ps_nxt[d] = ps
                        nc.tensor.matmul(ps[:], bsb[:, d], mask_sb[:],
                                         start=True, stop=False, perf_mode=DR)
                        for m in range(NM):
                            ms = slice(m * 10, m * 10 + 10)
                            dst = ps[:, ms]
                            for k2 in range(kx2):
                                nc.tensor.matmul(
                                    dst,
                                    wx_sb[:, d, 2 * k2:2 * k2 + 2,
                                          m * 128:(m + 1) * 128],
                                    xsrc(d, k2, tt),
                                    start=False,
                                    stop=(t == 0 and k2 == kx2 - 1
                                          and m == NM - 1),
                                    perf_mode=DR)

                    prefill(0, 0)
                    prefill(1, 0)
                    for t in range(WIDTH):
                        for d in range(2):
                            tt = t if d == 0 else WIDTH - 1 - t
                            ptt = (t - 1) if d == 0 else (WIDTH - t)
                            ps_cur[d] = ps_nxt[d]
                            ps = ps_cur[d]
                            # recurrent matmuls: [i,f,g] m-tiles first so the
                            # sigmoid chain starts earliest, o last.
                            if t > 0:
                                for m in range(NM):
                                    ms = slice(m * 10, m * 10 + 10)
                                    dst = ps[:, ms]
                                    for k2 in range(3):
                                        nc.tensor.matmul(
                                            dst,
                                            wh_sb[:, d, 2 * k2:2 * k2 + 2,
                                                  m * 128:(m + 1) * 128],
                                            hT_out[:, 6 * d + 2 * k2:
                                                   6 * d + 2 * k2 + 2,
                                                   ptt * 10:ptt * 10 + 10],
                                            start=False,
                                            stop=(k2 == 2 and m == NM - 1),
                                            perf_mode=DR)
                            if t + 1 < WIDTH:
                                # prefill next step for this dir while the
                                # other dir's Wh waits on its h semaphore
                                prefill(d, t + 1)
                            # ---- gate nonlinearity + state update ----
                            sg = sgp.tile([128, 240], BF16, tag=f"sg{d}",
                                          name=f"sg{l}{d}")
                            nc.scalar.activation(sg[:, 0:180],
                                                 ps[:, 0:180], SIG)
                            nc.scalar.activation(sg[:, 180:240],
                                                 ps[:, 180:240], SIG)
                            cf = scp.tile([128, 60], F32, tag=f"cf{d}",
                                          name=f"cf{l}{d}")
                            nc.gpsimd.tensor_tensor(cf[:], c[d][:],
                                                    sg[:, 60:120], MUL)
                            uu = scp.tile([128, 60], F32, tag=f"u{d}",
                                          name=f"u{l}{d}")
                            nc.vector._custom_dve(
                                OP_AB2_SUB, out=uu[:], in0=sg[:, 120:180],
                                in1=sg[:, 0:60])
                            nc.gpsimd.tensor_tensor(c[d][:], cf[:],
                                                    uu[:], ADD)
                            y0 = scp.tile([128, 60], F32, tag=f"y0{d}",
                                          name=f"y0{l}{d}")
                            nc.vector._custom_dve(
                                OP_TANH_SEED, out=y0[:], in0=c[d][:],
                                s0=14.0, s1=133.0, imm2=-0.23549792)
                            y1 = scp.tile([128, 60], F32, tag=f"y1{d}",
                                          name=f"y1{l}{d}")
                            nc.vector._custom_dve(
                                OP_TANH_NR, out=y1[:], in0=c[d][:],
                                in1=y0[:], s0=14.0, s1=133.0, imm2=2.0)
                            tn = scp.tile([128, 60], F32, tag=f"tn{d}",
                                          name=f"tn{l}{d}")
                            nc.vector._custom_dve(
                                OP_TANH_NUM, out=tn[:], in0=c[d][:],
                                in1=y1[:], s0=105.0, s1=945.0,
                                imm2=1.0 / 15.0)
                            so = sg[:, 180:240]
                            nc.vector._custom_dve(
                                OP_HMUL_CLAMP,
                                out=hT_out[:, 6 * d:6 * d + 6,
                                           tt * 10:tt * 10 + 10],
                                in0=so, in1=tn[:], s0=-1.0)
                            if stag:
                                blk = tt // 10
                                if stg[d] is None:
                                    stg[d] = stp.tile([128, 6, 100], BF16,
                                                      tag=f"st{d}",
                                                      name=f"st{d}")
                                tcl = scp.tile([128, 60], BF16,
                                               tag=f"tcl{d}",
                                               name=f"tcl{l}{d}")
                                nc.gpsimd.tensor_scalar(
                                    tcl[:], tn[:], 1.0, -1.0,
                                    mybir.AluOpType.min,
                                    mybir.AluOpType.max)
                                nc.gpsimd.tensor_tensor(
                                    stg[d][:, :, (tt % 10) * 10:
                                           (tt % 10) * 10 + 10],
                                    so.rearrange("p (a b) -> p a b", b=10),
                                    tcl[:].rearrange(
                                        "p (a b) -> p a b", b=10),
                                    MUL)
                                done = (tt % 10 == 9) if d == 0 else \
                                       (tt % 10 == 0)
                                if done:
                                    nc.sync.dma_start(
                                        h1_d[:, d, :,
                                             blk * 100:blk * 100 + 100],
                                        stg[d][:])
                                    stg[d] = None
                        if post_iter is not None:
                            post_iter(t)

            def load_w(pool, dram, kt, name):
                w = pool.tile([128, 2, kt, G], FP8, name=name)
                for d in range(2):
                    for k in range(kt):
                        nc.sync.dma_start(
                            w[:, d, k], dram[d, k * 128:(k + 1) * 128, :])
                return w

            with tc.tile_pool(name="wx1p", bufs=1) as wx1p:
                with tc.tile_pool(name="w0p", bufs=1) as w0p:
                    xT_sb = w0p.tile([128, 6, ROWS], FP8, name="xT")
                    for k in range(6):
                        nc.sync.dma_start(xT_sb[:, k],
                                          xT_d[k * 128:(k + 1) * 128, :])
                    wx0_sb = load_w(w0p, wx0_d, 6, "wx0")
                    wh0_sb = load_w(w0p, wh0_d, 6, "wh0")
                    # preload layer-1 input weights during recur0
                    wx1_sb = load_w(wx1p, wx1_d, 12, "wx1")
                    recur(0, wx0_sb, wh0_sb, bT_sb[0],
                          lambda d, k2, tt:
                          xT_sb[:, 2 * k2:2 * k2 + 2, tt * 10:tt * 10 + 10],
                          3, hT0)
                with tc.tile_pool(name="w1p", bufs=1) as w1p, \
                     tc.tile_pool(name="lwp", bufs=1) as lwp, \
                     tc.tile_pool(name="h1m", bufs=2) as h1mp, \
                     tc.tile_pool(name="lpp", bufs=3, space="PSUM") as lpp, \
                     tc.tile_pool(name="lop", bufs=2) as lop:
                    wh1_sb = load_w(w1p, wh1_d, 6, "wh1")
                    hT1 = w1p.tile([128, 12, ROWS], FP8, name="hT1")

                    # ---- final linear: y = h1cat @ linW + linb,
                    # emitted middle-out as h1 row-tiles complete ----
                    lw = lwp.tile([128, 12, DIM], BF16)
                    for k in range(12):
                        nc.sync.dma_start(
                            lw[:, k], linw_d[k * 128:(k + 1) * 128, :])
                    lbsb = lwp.tile([1, DIM], F32)
                    nc.sync.dma_start(lbsb[:], linb_d[None, :])
                    lbb = lwp.tile([128, DIM], F32)
                    for n in range(2):
                        ns = slice(n * 384, (n + 1) * 384)
                        bps = lpp.tile([128, 384], F32, tag="lp")
                        nc.tensor.matmul(bps[:], ones[:], lbsb[:, ns],
                                         start=True, stop=True)
                        nc.vector.tensor_copy(lbb[:, ns], bps[:])

                    def lin_tile_items(m):
                        mr = _mrows(m)
                        mc = m * 128
                        box = {}

                        def dma_in():
                            box["h1m"] = h1mp.tile([128, 12, 128], BF16,
                                                   tag="h1m", name="h1m")
                            nc.sync.dma_start(
                                box["h1m"][:, :, :mr],
                                h1_d[:, :, :, mc:mc + mr].rearrange(
                                    "p a b r -> p (a b) r"))
                            box["lo"] = lop.tile([128, DIM], F32, tag="lo",
                                                 name="lo")

                        items = [dma_in]
                        for n in range(2):
                            ns = slice(n * 384, (n + 1) * 384)

                            def open_ps(n=n, ns=ns):
                                box[f"ps{n}"] = lpp.tile([mr, 384], F32,
                                                         tag="lp", name="lps")

                            items.append(open_ps)
                            for kk in range(0, 12, 3):
                                def mms(n=n, ns=ns, kk=kk):
                                    for k in range(kk, kk + 3):
                                        nc.tensor.matmul(
                                            box[f"ps{n}"][:],
                                            box["h1m"][:, k, :mr],
                                            lw[:, k, ns],
                                            start=(k == 0), stop=(k == 11))
                                items.append(mms)

                            def drain(n=n, ns=ns):
                                nc.vector.tensor_tensor(
                                    box["lo"][:mr, ns], box[f"ps{n}"][:],
                                    lbb[:mr, ns], ADD)
                            items.append(drain)

                        def dma_out():
                            nc.sync.dma_start(out_d[mc:mc + mr, :],
                                              box["lo"][:mr])
                        items.append(dma_out)
                        return items

                    # readiness: tile m needs fwd h1 blocks flushed through
                    # its last row and bwd blocks through its first row
                    sched = {}
                    for m in range(MT):
                        b0 = (128 * m) // 100
                        b1 = (128 * m + _mrows(m) - 1) // 100
                        t_m = max(10 * b1 + 9, WIDTH - 1 - 10 * b0)
                        sched.setdefault(min(t_m, WIDTH - 1), []).append(m)
                    work = []

                    def post_iter(t):
                        for m in sched.get(t, []):
                            work.extend(lin_tile_items(m))
                        for _ in range(4):
                            if work:
                                work.pop(0)()

                    recur(1, wx1_sb, wh1_sb, bT_sb[1],
                          lambda d, k2, tt:
                          hT0[:, 2 * k2:2 * k2 + 2, tt * 10:tt * 10 + 10],
                          6, hT1, stag=True, post_iter=post_iter)
                    while work:
                        work.pop(0)()

    nc.compile()
    return nc


def _reorder_gates(w):
    """[i f g o] -> [i f 2g o] along last axis (size 4H)."""
    i, f, g, o = np.split(np.asarray(w, np.float32), 4, axis=-1)
    return np.concatenate([i, f, 2.0 * g, o], axis=-1)


def kernel(x, Wx0f, Wh0f, b0f, Wx0b, Wh0b, b0b,
           Wx1f, Wh1f, b1f, Wx1b, Wh1b, b1b, lin_W, lin_b):
    x = np.asarray(x, dtype=np.float32)
    # frame: (B, C, T) -> (NSEQ, DIM, WIDTH)
    tgt = (NFR - 1) * STRIDE + WIDTH
    xp = np.zeros((B, DIM, tgt), dtype=np.float32)
    xp[:, :, :T] = x
    frames = np.stack([xp[:, :, i:i + WIDTH]
                       for i in range(0, tgt - WIDTH + 1, STRIDE)], axis=1)
    xf = frames.reshape(NSEQ, DIM, WIDTH)

    def prepw(wf, wb):
        return np.ascontiguousarray(np.stack(
            [_reorder_gates(wf), _reorder_gates(wb)])).astype(FP8_NP)

    def prepb(bf, bb_):
        return np.stack([_reorder_gates(bf).reshape(12, 2, 128),
                         _reorder_gates(bb_).reshape(12, 2, 128)])

    mask = np.zeros((12, 2, NM * 10), dtype=np.float32)
    for j in range(12):
        for r in range(2):
            m = 2 * j + r
            mask[j, r, m * 10:(m + 1) * 10] = 1.0

    wx0 = prepw(Wx0f, Wx0b)
    wh0 = prepw(Wh0f, Wh0b)
    wx1 = prepw(Wx1f, Wx1b)
    wh1 = prepw(Wh1f, Wh1b)
    bT = np.ascontiguousarray(np.stack(
        [prepb(b0f, b0b), prepb(b1f, b1b)])).astype(FP8_NP)
    linw = np.ascontiguousarray(np.asarray(lin_W, np.float32)).astype(BF16_NP)
    linb = np.ascontiguousarray(np.asarray(lin_b, np.float32))

    if "nc" not in _CACHE:
        _CACHE["nc"] = _build_program()
    nc = _CACHE["nc"]

    in_maps = []
    for cc in range(NCORES):
        shard = xf[cc * SEQ_PC:(cc + 1) * SEQ_PC]       # (10, 768, 200)
        xT = shard.transpose(1, 2, 0).reshape(DIM, ROWS)  # col = t*10 + s
        in_maps.append({"xT": np.ascontiguousarray(xT).astype(FP8_NP),
                        "wx0": wx0, "wh0": wh0, "wx1": wx1, "wh1": wh1,
                        "bT": bT, "mask": mask.astype(FP8_NP),
                        "linw": linw, "linb": linb})
    _CACHE["in_maps"] = in_maps

    res = run_bass_kernel_spmd(nc, in_maps, list(range(NCORES)))
    outs = [np.asarray(res.results[cc]["out"], np.float32)
            .reshape(WIDTH, SEQ_PC, DIM).transpose(1, 0, 2)
            for cc in range(NCORES)]                     # (10, 200, 768)
    y = np.concatenate(outs, axis=0)                     # (80, 200, 768)
    y = y.transpose(0, 2, 1).reshape(B, NFR, DIM, WIDTH)

    limit = STRIDE // 2
    parts = [y[:, 0, :, :-limit]]
    for k in range(1, NFR - 1):
        parts.append(y[:, k, :, limit:-limit])
    parts.append(y[:, NFR - 1, :, limit:])
    yc = np.concatenate(parts, axis=-1)[:, :, :T]        # (4, 768, 2000)
    return (yc + x).astype(np.float32)


# revision 20
# speedup vs baseline: 1.0037x; 1.0037x over previous
"""Bass/Trainium2 kernel for framed 2-layer BiLSTM (nn_BLSTM).

Data-parallel over the 80 framed sequences: 10 per core on 8 NeuronCores.

Restructured recurrence (vs the staged-projection baseline): the input
projection Wx*x_t, bias, and recurrent Wh*h_{t-1} all accumulate directly
into one [128 x 240] PSUM tile per direction per step (transposed layout:
partitions = gate channels mod 128, columns = m-tile x sequence). All
matmuls are fp8 DoubleRow (K=256 per instruction). Gate order is host-
reordered to [i, f, g, o] with the g block pre-scaled by 2 so ONE sigmoid
over [i,f,2g] yields sigma(i), sigma(f), and sigma(2g) = (tanh(g)+1)/2;
tanh(g) is then reconstructed on the DVE via c = sf*c + 2*(si*s2g) - si.
No xw staging in DRAM, no PE injects, no projection drains.

Layer-1 consumes layer-0's fp8 h buffer directly as its Wx input. The
final linear consumes a bf16 copy of h1 streamed through DRAM (SBUF
chunks -> DRAM -> row tiles), keeping fp8 out of the output path.
"""
import sys
import numpy as np

sys.path.insert(0, "/opt/trn_rl_repo")

import ml_dtypes  # noqa: E402
import concourse.bass as bass  # noqa: E402
import concourse.mybir as mybir  # noqa: E402
from concourse import bacc  # noqa: E402
from concourse.tile import TileContext  # noqa: E402
from concourse.bass_utils import run_bass_kernel_spmd  # noqa: E402

# ---- custom DVE ops: fused tanh(g) reconstruction and a 3-op tanh(c) ----
# tanh(x) ~ clamp(x*(x^4+105x^2+945) / (15*((x^2+14)^2-133))), Pade(5,4)
# with a bitwise-NOT reciprocal seed + one Newton step (max abs err ~3.4e-3).
import concourse.dve_ops as _dvo  # noqa: E402
from concourse.dve_spec import (  # noqa: E402
    Spec, Src0, Src1, C0, C1, C2, One, AluOp, Bin, sq, maxx, minn, lower,
    _has_src1)
from concourse.dve_uop import DveOpSpec  # noqa: E402


def _register_dve_op(name, body, reference):
    for op in _dvo.OPS:
        if op.name == name:
            return op
    spec = Spec(body=body, reference=reference)
    row = max(_dvo._SUB_OPCODE_FOR_NAME.values()) + 1
    assert row < 0x20
    _dvo._SUB_OPCODE_FOR_NAME[name] = row
    sha = DveOpSpec(name=name, opcode=row, uops=lower(spec, ver="v3"),
                    rd1_en=_has_src1(spec)).sha("v3")
    op = _dvo.DveOp(name, spec, subdim=False, uops_sha={"v3": sha})
    _dvo.OPS.append(op)
    _dvo.CUSTOM_DVE_SPECS[name] = spec
    return op


def _bnot(v):
    return (~v.view(np.int32)).view(np.float32)


_x2 = sq(Src0)
_dq = sq(_x2 + C0) - C1
OP_TANH_SEED = _register_dve_op(
    "TANH_SEED_ANT", Bin(AluOp.BITWISE_NOT, _dq, _dq) * C2,
    lambda in0, in1, c0, c1, c2:
    _bnot(((in0 * in0 + c0) ** 2 - c1).astype(np.float32)) * c2)
OP_TANH_NR = _register_dve_op(
    "TANH_NR_ANT", Src1 * (C2 - (sq(sq(Src0) + C0) - C1) * Src1),
    lambda in0, in1, c0, c1, c2:
    in1 * (c2 - ((in0 * in0 + c0) ** 2 - c1) * in1))
OP_TANH_NUM = _register_dve_op(
    "TANH_NUM_ANT", (((sq(Src0) + C0) * sq(Src0) + C1) * Src0) * Src1 * C2,
    lambda in0, in1, c0, c1, c2:
    ((in0 * in0 + c0) * (in0 * in0) + c1) * in0 * in1 * c2)
OP_HMUL_CLAMP = _register_dve_op(
    "HMUL_CLAMP_ANT", Src0 * minn(maxx(Src1, C0), One),
    lambda in0, in1, c0, c1, c2: in0 * np.minimum(np.maximum(in1, c0), 1.0))
OP_AB2_SUB = _register_dve_op(
    "AB2_SUB_ANT", ((Src0 + Src0) - One) * Src1,
    lambda in0, in1, c0, c1, c2: (2.0 * in0 - 1.0) * in1)

F32 = mybir.dt.float32
BF16 = mybir.dt.bfloat16
FP8 = mybir.dt.float8e4
BF16_NP = ml_dtypes.bfloat16
FP8_NP = ml_dtypes.float8_e4m3
DR = mybir.MatmulPerfMode.DoubleRow

DIM = 768
H = 768
G = 4 * H            # 3072, gate order reordered to [i, f, 2g, o]
B, T = 4, 2000
WIDTH, STRIDE = 200, 100
NFR = 20             # frames per batch element
NSEQ = B * NFR       # 80
NCORES = 8
SEQ_PC = NSEQ // NCORES   # 10
ROWS = SEQ_PC * WIDTH     # 2000 rows per core, row = t*10 + s (t-major)
MT = (ROWS + 127) // 128  # 16 row m-tiles for the final linear
NM = 24                   # gate m-tiles

SIG = mybir.ActivationFunctionType.Sigmoid
TANH = mybir.ActivationFunctionType.Tanh
MUL = mybir.AluOpType.mult
ADD = mybir.AluOpType.add
SUB = mybir.AluOpType.subtract

_CACHE = {}


def _mrows(m):
    return min(128, ROWS - m * 128)


def _build_program():
    nc = bacc.Bacc("TRN2", target_bir_lowering=False, debug=False,
                   num_devices=NCORES)

    xT_d = nc.declare_dram_parameter("xT", [DIM, ROWS], FP8, isOutput=False)
    wx0_d = nc.declare_dram_parameter("wx0", [2, DIM, G], FP8, isOutput=False)
    wh0_d = nc.declare_dram_parameter("wh0", [2, H, G], FP8, isOutput=False)
    wx1_d = nc.declare_dram_parameter("wx1", [2, 2 * H, G], FP8,
                                      isOutput=False)
    wh1_d = nc.declare_dram_parameter("wh1", [2, H, G], FP8, isOutput=False)
    bT_d = nc.declare_dram_parameter("bT", [2, 2, 12, 2, 128], FP8,
                                     isOutput=False)
    mask_d = nc.declare_dram_parameter("mask", [12, 2, NM * 10], FP8,
                                       isOutput=False)
    linw_d = nc.declare_dram_parameter("linw", [2 * H, DIM], BF16,
                                       isOutput=False)
    linb_d = nc.declare_dram_parameter("linb", [DIM], F32, isOutput=False)
    out_d = nc.declare_dram_parameter("out", [ROWS, DIM], F32, isOutput=True)

    # bf16 h1 staging for the final linear: [p, dir, k, row]
    h1_d = nc.dram_tensor("h1s", [128, 2, 6, ROWS], BF16)

    with TileContext(nc) as tc:
        with tc.tile_pool(name="const", bufs=1) as constp, \
             tc.tile_pool(name="hbp", bufs=1) as hbp, \
             tc.tile_pool(name="cstp", bufs=1) as cstp:
            ones = constp.tile([1, 128], F32)
            nc.vector.memset(ones[:], 1.0)
            bT_sb = [constp.tile([12, 2, 2, 128], FP8, name=f"bT{l}")
                     for l in range(2)]
            for l in range(2):
                for d in range(2):
                    nc.sync.dma_start(bT_sb[l][:, d], bT_d[l, d])
            mask_sb = constp.tile([12, 2, NM * 10], FP8)
            nc.sync.dma_start(mask_sb[:], mask_d[:])

            hT0 = hbp.tile([128, 12, ROWS], FP8, name="hT0")

            # ---- one recurrence layer, both dirs interleaved ----
            # xsrc(d, k2, tt) -> [128, 2, 10] moving AP for the input
            # projection; kx2 = number of fp8-DoubleRow K pairs of the input.
            def recur(l, wx_sb, wh_sb, bsb, xsrc, kx2, hT_out, stag=False,
                      post_iter=None):
                with tc.tile_pool(name=f"gp{l}", bufs=2, space="PSUM") as gpp,\
                     tc.tile_pool(name=f"sg{l}", bufs=2) as sgp, \
                     tc.tile_pool(name=f"sc{l}", bufs=2) as scp, \
                     tc.tile_pool(name=f"st{l}", bufs=2) as stp:
                    c = [cstp.tile([128, 60], F32, name=f"c{l}{d}")
                         for d in range(2)]
                    for d in range(2):
                        nc.vector.memset(c[d][:], 0.0)
                    ps_cur = [None, None]
                    ps_nxt = [None, None]
                    stg = [None, None]

                    def prefill(d, t):
                        tt = t if d == 0 else WIDTH - 1 - t
                        ps = gpp.tile([128, 240], F32, tag=f"g{d}",
                                      name=f"g{l}{d}")
                        ps_nxt[d] = ps
                        nc.tensor.matmul(ps[:], bsb[:, d], mask_sb[:],
                                         start=True, stop=False, perf_mode=DR)
                        for m in range(NM):
                            ms = slice(m * 10, m * 10 + 10)
                            dst = ps[:, ms]
                            for k2 in range(kx2):
                                nc.tensor.matmul(
                                    dst,
                                    wx_sb[:, d, 2 * k2:2 * k2 + 2,
                                          m * 128:(m + 1) * 128],
                                    xsrc(d, k2, tt),
                                    start=False,
                                    stop=(t == 0 and k2 == kx2 - 1
                                          and m == NM - 1),
                                    perf_mode=DR)

                    prefill(0, 0)
                    prefill(1, 0)
                    for t in range(WIDTH):
                        st_ = {}
                        for d in range(2):
                            tt = t if d == 0 else WIDTH - 1 - t
                            ptt = (t - 1) if d == 0 else (WIDTH - t)
                            ps_cur[d] = ps_nxt[d]
                            ps = ps_cur[d]
                            # recurrent matmuls: [i,f,g] m-tiles first so the
                            # sigmoid chain starts earliest, o last.
                            if t > 0:
                                for m in range(NM):
                                    ms = slice(m * 10, m * 10 + 10)
                                    dst = ps[:, ms]
                                    for k2 in range(3):
                                        nc.tensor.matmul(
                                            dst,
                                            wh_sb[:, d, 2 * k2:2 * k2 + 2,
                                                  m * 128:(m + 1) * 128],
                                            hT_out[:, 6 * d + 2 * k2:
                                                   6 * d + 2 * k2 + 2,
                                                   ptt * 10:ptt * 10 + 10],
                                            start=False,
                                            stop=(k2 == 2 and m == NM - 1),
                                            perf_mode=DR)
                            if t + 1 < WIDTH:
                                # prefill next step for this dir while the
                                # other dir's Wh waits on its h semaphore
                                prefill(d, t + 1)
                            # ---- gate nonlinearity + state update ----
                            sg = sgp.tile([128, 240], BF16, tag=f"sg{d}",
                                          name=f"sg{l}{d}")
                            nc.scalar.activation(sg[:, 0:180],
                                                 ps[:, 0:180], SIG)
                            nc.scalar.activation(sg[:, 180:240],
                                                 ps[:, 180:240], SIG)
                            st_[d] = (sg, tt)
                        # elementwise chains, both dirs interleaved op-by-op
                        # so each dir's dependency gaps are filled by the
                        # other dir's work on the in-order DVE queue
                        cf_, uu_, tn_ = {}, {}, {}
                        for d in range(2):
                            sg, tt = st_[d]
                            cf_[d] = scp.tile([128, 60], F32, tag=f"cf{d}",
                                              name=f"cf{l}{d}")
                            nc.gpsimd.tensor_tensor(cf_[d][:], c[d][:],
                                                    sg[:, 60:120], MUL)
                        for d in range(2):
                            sg, tt = st_[d]
                            uu_[d] = scp.tile([128, 60], F32, tag=f"u{d}",
                                              name=f"u{l}{d}")
                            nc.vector._custom_dve(
                                OP_AB2_SUB, out=uu_[d][:],
                                in0=sg[:, 120:180], in1=sg[:, 0:60])
                        for d in range(2):
                            nc.vector.tensor_tensor(c[d][:], cf_[d][:],
                                                    uu_[d][:], ADD)
                        y0_, y1_ = {}, {}
                        for d in range(2):
                            y0_[d] = scp.tile([128, 60], F32, tag=f"y0{d}",
                                              name=f"y0{l}{d}")
                            nc.vector._custom_dve(
                                OP_TANH_SEED, out=y0_[d][:], in0=c[d][:],
                                s0=14.0, s1=133.0, imm2=-0.23549792)
                        for d in range(2):
                            y1_[d] = scp.tile([128, 60], F32, tag=f"y1{d}",
                                              name=f"y1{l}{d}")
                            nc.vector._custom_dve(
                                OP_TANH_NR, out=y1_[d][:], in0=c[d][:],
                                in1=y0_[d][:], s0=14.0, s1=133.0, imm2=2.0)
                        for d in range(2):
                            tn_[d] = scp.tile([128, 60], F32, tag=f"tn{d}",
                                              name=f"tn{l}{d}")
                            nc.vector._custom_dve(
                                OP_TANH_NUM, out=tn_[d][:], in0=c[d][:],
                                in1=y1_[d][:], s0=105.0, s1=945.0,
                                imm2=1.0 / 15.0)
                        for d in range(2):
                            sg, tt = st_[d]
                            so = sg[:, 180:240]
                            nc.vector._custom_dve(
                                OP_HMUL_CLAMP,
                                out=hT_out[:, 6 * d:6 * d + 6,
                                           tt * 10:tt * 10 + 10],
                                in0=so, in1=tn_[d][:], s0=-1.0)
                            if stag:
                                blk = tt // 10
                                if stg[d] is None:
                                    stg[d] = stp.tile([128, 6, 100], BF16,
                                                      tag=f"st{d}",
                                                      name=f"st{d}")
                                tcl = scp.tile([128, 60], BF16,
                                               tag=f"tcl{d}",
                                               name=f"tcl{l}{d}")
                                nc.gpsimd.tensor_scalar(
                                    tcl[:], tn_[d][:], 1.0, -1.0,
                                    mybir.AluOpType.min,
                                    mybir.AluOpType.max)
                                nc.gpsimd.tensor_tensor(
                                    stg[d][:, :, (tt % 10) * 10:
                                           (tt % 10) * 10 + 10],
                                    so.rearrange("p (a b) -> p a b", b=10),
                                    tcl[:].rearrange(
                                        "p (a b) -> p a b", b=10),
                                    MUL)
                                done = (tt % 10 == 9) if d == 0 else \
                                       (tt % 10 == 0)
                                if done:
                                    nc.sync.dma_start(
                                        h1_d[:, d, :,
                                             blk * 100:blk * 100 + 100],
                                        stg[d][:])
                                    stg[d] = None
                        if post_iter is not None:
                            post_iter(t)

            def load_w(pool, dram, kt, name):
                w = pool.tile([128, 2, kt, G], FP8, name=name)
                for d in range(2):
                    for k in range(kt):
                        nc.sync.dma_start(
                            w[:, d, k], dram[d, k * 128:(k + 1) * 128, :])
                return w

            with tc.tile_pool(name="wx1p", bufs=1) as wx1p:
                with tc.tile_pool(name="w0p", bufs=1) as w0p:
                    xT_sb = w0p.tile([128, 6, ROWS], FP8, name="xT")
                    for k in range(6):
                        nc.sync.dma_start(xT_sb[:, k],
                                          xT_d[k * 128:(k + 1) * 128, :])
                    wx0_sb = load_w(w0p, wx0_d, 6, "wx0")
                    wh0_sb = load_w(w0p, wh0_d, 6, "wh0")
                    # preload layer-1 input weights during recur0
                    wx1_sb = load_w(wx1p, wx1_d, 12, "wx1")
                    recur(0, wx0_sb, wh0_sb, bT_sb[0],
                          lambda d, k2, tt:
                          xT_sb[:, 2 * k2:2 * k2 + 2, tt * 10:tt * 10 + 10],
                          3, hT0)
                with tc.tile_pool(name="w1p", bufs=1) as w1p, \
                     tc.tile_pool(name="lwp", bufs=1) as lwp, \
                     tc.tile_pool(name="h1m", bufs=2) as h1mp, \
                     tc.tile_pool(name="lpp", bufs=3, space="PSUM") as lpp, \
                     tc.tile_pool(name="lop", bufs=2) as lop:
                    wh1_sb = load_w(w1p, wh1_d, 6, "wh1")
                    hT1 = w1p.tile([128, 12, ROWS], FP8, name="hT1")

                    # ---- final linear: y = h1cat @ linW + linb,
                    # emitted middle-out as h1 row-tiles complete ----
                    lw = lwp.tile([128, 12, DIM], BF16)
                    for k in range(12):
                        nc.sync.dma_start(
                            lw[:, k], linw_d[k * 128:(k + 1) * 128, :])
                    lbsb = lwp.tile([1, DIM], F32)
                    nc.sync.dma_start(lbsb[:], linb_d[None, :])
                    lbb = lwp.tile([128, DIM], F32)
                    for n in range(2):
                        ns = slice(n * 384, (n + 1) * 384)
                        bps = lpp.tile([128, 384], F32, tag="lp")
                        nc.tensor.matmul(bps[:], ones[:], lbsb[:, ns],
                                         start=True, stop=True)
                        nc.vector.tensor_copy(lbb[:, ns], bps[:])

                    def lin_tile_items(m):
                        mr = _mrows(m)
                        mc = m * 128
                        box = {}

                        def dma_in():
                            box["h1m"] = h1mp.tile([128, 12, 128], BF16,
                                                   tag="h1m", name="h1m")
                            nc.sync.dma_start(
                                box["h1m"][:, :, :mr],
                                h1_d[:, :, :, mc:mc + mr].rearrange(
                                    "p a b r -> p (a b) r"))
                            box["lo"] = lop.tile([128, DIM], F32, tag="lo",
                                                 name="lo")

                        items = [dma_in]
                        for n in range(2):
                            ns = slice(n * 384, (n + 1) * 384)

                            def open_ps(n=n, ns=ns):
                                box[f"ps{n}"] = lpp.tile([mr, 384], F32,
                                                         tag="lp", name="lps")

                            items.append(open_ps)
                            for kk in range(0, 12, 3):
                                def mms(n=n, ns=ns, kk=kk):
                                    for k in range(kk, kk + 3):
                                        nc.tensor.matmul(
                                            box[f"ps{n}"][:],
                                            box["h1m"][:, k, :mr],
                                            lw[:, k, ns],
                                            start=(k == 0), stop=(k == 11))
                                items.append(mms)

                            def drain(n=n, ns=ns):
                                nc.vector.tensor_tensor(
                                    box["lo"][:mr, ns], box[f"ps{n}"][:],
                                    lbb[:mr, ns], ADD)
                            items.append(drain)

                        def dma_out():
                            nc.sync.dma_start(out_d[mc:mc + mr, :],
                                              box["lo"][:mr])
                        items.append(dma_out)
                        return items

                    # readiness: tile m needs fwd h1 blocks flushed through
                    # its last row and bwd blocks through its first row
                    sched = {}
                    for m in range(MT):
                        b0 = (128 * m) // 100
                        b1 = (128 * m + _mrows(m) - 1) // 100
                        t_m = max(10 * b1 + 9, WIDTH - 1 - 10 * b0)
                        sched.setdefault(min(t_m, WIDTH - 1), []).append(m)
                    work = []

                    def post_iter(t):
                        for m in sched.get(t, []):
                            work.extend(lin_tile_items(m))
                        for _ in range(3):
                            if work:
                                work.pop(0)()

                    recur(1, wx1_sb, wh1_sb, bT_sb[1],
                          lambda d, k2, tt:
                          hT0[:, 2 * k2:2 * k2 + 2, tt * 10:tt * 10 + 10],
                          6, hT1, stag=True, post_iter=post_iter)
                    while work:
                        work.pop(0)()

    nc.compile()
    return nc


def _reorder_gates(w):
    """[i f g o] -> [i f 2g o] along last axis (size 4H)."""
    i, f, g, o = np.split(np.asarray(w, np.float32), 4, axis=-1)
    return np.concatenate([i, f, 2.0 * g, o], axis=-1)


def kernel(x, Wx0f, Wh0f, b0f, Wx0b, Wh0b, b0b,
           Wx1f, Wh1f, b1f, Wx1b, Wh1b, b1b, lin_W, lin_b):
    x = np.asarray(x, dtype=np.float32)
    # frame: (B, C, T) -> (NSEQ, DIM, WIDTH)
    tgt = (NFR - 1) * STRIDE + WIDTH
    xp = np.zeros((B, DIM, tgt), dtype=np.float32)
    xp[:, :, :T] = x
    frames = np.stack([xp[:, :, i:i + WIDTH]
                       for i in range(0, tgt - WIDTH + 1, STRIDE)], axis=1)
    xf = frames.reshape(NSEQ, DIM, WIDTH)

    def prepw(wf, wb):
        return np.ascontiguousarray(np.stack(
            [_reorder_gates(wf), _reorder_gates(wb)])).astype(FP8_NP)

    def prepb(bf, bb_):
        return np.stack([_reorder_gates(bf).reshape(12, 2, 128),
                         _reorder_gates(bb_).reshape(12, 2, 128)])

    mask = np.zeros((12, 2, NM * 10), dtype=np.float32)
    for j in range(12):
        for r in range(2):
            m = 2 * j + r
            mask[j, r, m * 10:(m + 1) * 10] = 1.0

    wx0 = prepw(Wx0f, Wx0b)
    wh0 = prepw(Wh0f, Wh0b)
    wx1 = prepw(Wx1f, Wx1b)
    wh1 = prepw(Wh1f, Wh1b)
    bT = np.ascontiguousarray(np.stack(
        [prepb(b0f, b0b), prepb(b1f, b1b)])).astype(FP8_NP)
    linw = np.ascontiguousarray(np.asarray(lin_W, np.float32)).astype(BF16_NP)
    linb = np.ascontiguousarray(np.asarray(lin_b, np.float32))

    if "nc" not in _CACHE:
        _CACHE["nc"] = _build_program()
    nc = _CACHE["nc"]

    in_maps = []
    for cc in range(NCORES):
        shard = xf[cc * SEQ_PC:(cc + 1) * SEQ_PC]       # (10, 768, 200)
        xT = shard.transpose(1, 2, 0).reshape(DIM, ROWS)  # col = t*10 + s
        in_maps.append({"xT": np.ascontiguousarray(xT).astype(FP8_NP),
                        "wx0": wx0, "wh0": wh0, "wx1": wx1, "wh1": wh1,
                        "bT": bT, "mask": mask.astype(FP8_NP),
                        "linw": linw, "linb": linb})
    _CACHE["in_maps"] = in_maps

    res = run_bass_kernel_spmd(nc, in_maps, list(range(NCORES)))
    outs = [np.asarray(res.results[cc]["out"], np.float32)
            .reshape(WIDTH, SEQ_PC, DIM).transpose(1, 0, 2)
            for cc in range(NCORES)]                     # (10, 200, 768)
    y = np.concatenate(outs, axis=0)                     # (80, 200, 768)
    y = y.transpose(0, 2, 1).reshape(B, NFR, DIM, WIDTH)

    limit = STRIDE // 2
    parts = [y[:, 0, :, :-limit]]
    for k in range(1, NFR - 1):
        parts.append(y[:, k, :, limit:-limit])
    parts.append(y[:, NFR - 1, :, limit:])
    yc = np.concatenate(parts, axis=-1)[:, :, :T]        # (4, 768, 2000)
    return (yc + x).astype(np.float32)
